# revision 1
# baseline (speedup 1.0000x reference)
"""Trainium2 Bass kernel for nn_EquivariantThreeHopGINE (4-layer GINE + LN GNN).

Self-contained: takes the FULL inputs of reference.setup_inputs(), shards nodes
across 8 NeuronCores, runs a Bass/Tile kernel via run_bass_kernel_spmd, and
returns the FULL [200000, 256] output.

Math (per reference.py):
  h0 = concat(7 embedding lookups)(atom_inputs) @ W0 + b0
  per GINE layer i: m = relu(h + c_i) with c_i = We_i + be_i (edge MLP of
  all-ones edge_attr); agg[v] = sum_{u->v} m[u] over symmetrized edges;
  h = LayerNorm((h + agg) @ Wn_i + bn_i) * g_i + bt_i
  out = h @ W1 + b1

Device mapping:
  - AtomEmbedding+linear_0 folded into a per-unique-atom-row LUT (host builds
    LUT = embed(unique rows) @ W0 + b0; device does one dma_gather).
  - Nodes sharded contiguously (25000/core, padded to 25088). Each layer:
    AllGather m -> m_full; dma_gather source rows (grouped in 32768-row blocks
    for int16 indices); dma_scatter_add into the agg buffer (initialized with
    h by the previous layer's node update); node update does the weight matmul
    feature-major via PE transposes, computes LN stats node-major, and fuses
    the next layer's h-init and m = relu(...) production.
  - dma_scatter_add loses colliding updates within one call, so the host
    assigns each chunk duplicate-free destinations; chunks are padded to a
    core-uniform static grid with fake edges aimed at trash rows.
"""

import os
import numpy as np

import concourse.bass as bass
import concourse.mybir as mybir
import concourse.tile as tile
from concourse.bass_utils import run_bass_kernel_spmd
from concourse import library_config
from concourse.masks import make_identity

FP = mybir.dt.float32
I16 = mybir.dt.int16
AF = mybir.ActivationFunctionType
ALU = mybir.AluOpType

N = 200000
E = 400000
H = 256
LN_EPS = 1e-5
NCORE = 8
NLOC = N // NCORE            # 25000
NODE_TILE = 512
NPAD = ((NLOC + NODE_TILE - 1) // NODE_TILE) * NODE_TILE   # 25088
NFULL = NCORE * NPAD
BLK = 32768                  # int16 gather window (rows)
TRASH = 512                  # trash rows appended to agg for fake scatter dsts
CH_L0 = 1024
CH_EDGE = 1024               # target edge-chunk size (>1024 idx crashes SWDGE)


# ---------------------------------------------------------------------------
# toolchain workarounds for this container's walrus build
# ---------------------------------------------------------------------------

def _finalize_bir(nc):
    """Split multi-waits (walrus here supports one embedded wait per
    instruction) and assemble InstISA subclass bytes (library reloads)."""
    for fn in nc.m.functions:
        for b in fn.blocks:
            newlist = []
            n_split = 0
            for ins in b.instructions:
                si = ins.sync_info
                if si is not None and si.on_wait and len(si.on_wait) > 1:
                    waits = list(si.on_wait)
                    for w in waits[:-1]:
                        newlist.append(mybir.InstNoOp(
                            name=f"{ins.name}-ws{n_split}",
                            engine=ins.engine,
                            sync_info=mybir.SyncInfo(on_wait=[w], on_update=[]),
                        ))
                        n_split += 1
                    ins.sync_info = mybir.SyncInfo(
                        on_wait=[waits[-1]], on_update=list(si.on_update or []))
                newlist.append(ins)
            b.instructions = newlist
    mybir.codegen_inst_isa_subclasses(nc)


def _wrap16(arr):
    """[n] int array (n % 16 == 0, -1 padding allowed) -> [128, n/16] int16
    SBUF layout for dma_gather/dma_scatter_add: element i at [i%16, i//16],
    replicated across the 8 groups of 16 partitions."""
    a = np.asarray(arr, np.int64)
    n = len(a)
    assert n % 16 == 0
    assert a.max(initial=0) < 32768 and a.min(initial=0) >= -1
    buf = a.astype(np.int16).reshape(n // 16, 16).T          # [16, n/16]
    return np.ascontiguousarray(np.tile(buf, (8, 1)))        # [128, n/16]


# ---------------------------------------------------------------------------
# host-side graph preprocessing
# ---------------------------------------------------------------------------

def _balanced_fill(loads, n_extra, cap):
    """How many extra items per bin to stay balanced; sum == n_extra."""
    nch = len(loads)
    fill = np.zeros(nch, np.int64)
    total = loads.sum() + n_extra
    desired = -(-total // nch)
    fill = np.clip(desired - loads, 0, cap - loads)
    # correct rounding drift
    diff = int(fill.sum() - n_extra)
    i = 0
    order = np.argsort(-(loads + fill))
    while diff > 0:
        c = order[i % nch]
        take = min(diff, int(fill[c]))
        fill[c] -= take
        diff -= take
        i += 1
    order = np.argsort(loads + fill)
    i = 0
    while diff < 0:
        c = order[i % nch]
        room = int(cap - loads[c] - fill[c])
        give = min(-diff, room)
        fill[c] += give
        diff += give
        i += 1
    assert fill.sum() == n_extra and (loads + fill <= cap).all()
    return fill


def _assign_chunks(dls, nch, size):
    """Assign each edge to one of nch chunks of capacity `size` such that no
    chunk holds two edges with the same destination."""
    n = len(dls)
    chunk_of = np.empty(n, np.int64)
    order = np.argsort(dls, kind="stable")
    dsorted = dls[order]
    uniq, start, counts = np.unique(dsorted, return_index=True,
                                    return_counts=True)
    loads = np.zeros(nch, np.int64)
    multi = np.nonzero(counts >= 2)[0]
    # big groups first so the m-least-loaded choice always succeeds
    for gi in multi[np.argsort(-counts[multi], kind="stable")]:
        m = int(counts[gi])
        st = int(start[gi])
        assert m <= nch
        sel = np.argpartition(loads, m - 1)[:m] if m < nch else np.arange(nch)
        chunk_of[order[st:st + m]] = sel
        loads[sel] += 1
    singles = np.nonzero(counts == 1)[0]
    pos = order[start[singles]]
    fill = _balanced_fill(loads, len(pos), size)
    chunk_of[pos] = np.repeat(np.arange(nch), fill)
    loads += fill
    assert (loads <= size).all()
    return chunk_of


def _prep_edges(src, dst, nloc, npad, nblk_rows, ncore, ch_edge, trash):
    """Returns (edge_grid, gidx_bycore, sidx_bycore).
    edge_grid: list of (block_idx, n_src_rows_in_block, [chunk_size]*nch) --
    identical for every core. gidx/sidx: per-core flat int arrays (len GTOT)."""
    s_all = np.concatenate([src, dst])
    d_all = np.concatenate([dst, src])
    nfull = ncore * npad
    nblocks = -(-nfull // nblk_rows)

    per_core = []
    n_kb = np.zeros((ncore, nblocks), np.int64)
    mult_kb = np.zeros((ncore, nblocks), np.int64)
    for k in range(ncore):
        sel = (d_all // nloc) == k
        sk = s_all[sel]
        dl = d_all[sel] - k * nloc
        grow = (sk // nloc) * npad + (sk % nloc)
        blk = grow // nblk_rows
        rows_rel = grow - blk * nblk_rows
        by_block = []
        for b in range(nblocks):
            m = blk == b
            rb, db = rows_rel[m], dl[m]
            n_kb[k, b] = len(rb)
            if len(rb):
                _, c = np.unique(db, return_counts=True)
                mult_kb[k, b] = c.max()
            by_block.append((rb, db))
        per_core.append(by_block)

    edge_grid = []
    for b in range(nblocks):
        maxn = int(n_kb[:, b].max())
        if maxn == 0:
            continue
        nch = max(-(-maxn // ch_edge), int(mult_kb[:, b].max()), 1)
        size = -(-(-(-maxn // nch)) // 128) * 128
        rows_span = min(nblk_rows, nfull - b * nblk_rows)
        edge_grid.append((b, rows_span, [size] * nch))

    gidx_bycore, sidx_bycore = [], []
    for k in range(ncore):
        gparts, sparts = [], []
        for (b, _, sizes) in edge_grid:
            rb, db = per_core[k][b]
            nch, size = len(sizes), sizes[0]
            chunk_of = _assign_chunks(db, nch, size) if len(db) else \
                np.empty(0, np.int64)
            for c in range(nch):
                m = chunk_of == c
                g = rb[m]
                s = db[m]
                nfake = size - len(g)
                assert 0 <= nfake <= trash, (k, b, c, nfake)
                gparts.append(np.concatenate(
                    [g, np.zeros(nfake, np.int64)]))
                sparts.append(np.concatenate(
                    [s, npad + np.arange(nfake)]))
        gidx_bycore.append(np.concatenate(gparts))
        sidx_bycore.append(np.concatenate(sparts))
    return edge_grid, gidx_bycore, sidx_bycore


# ---------------------------------------------------------------------------
# device program
# ---------------------------------------------------------------------------

def build_program(cfg):
    ncore = cfg["ncore"]
    nloc, npad, nfull = cfg["nloc"], cfg["npad"], cfg["nfull"]
    lutrows = cfg["lutrows"]
    l0_grid = cfg["l0_grid"]          # [(size, valid)]
    edge_grid = cfg["edge_grid"]      # [(block, rows_span, [sizes])]
    gtot = cfg["gtot"]
    blk_rows = cfg["blk_rows"]
    node_tile = cfg["node_tile"]
    trash = cfg["trash"]
    ncv = 12                          # packed per-feature const vectors

    nt = node_tile // 128             # node sub-blocks per tile
    n_tiles = npad // node_tile
    cmax = max(sz for (_, _, sizes) in edge_grid for sz in sizes) // 128
    cmax = max(cmax, max(sz for (sz, _) in l0_grid) // 128)

    nc = bass.Bass(num_devices=ncore)

    lut_in = nc.dram_tensor("lut", [lutrows, H], FP, kind="ExternalInput")
    codes_in = nc.dram_tensor("codes", [128, npad // 16], I16, kind="ExternalInput")
    gidx_in = nc.dram_tensor("gidx", [128, gtot // 16], I16, kind="ExternalInput")
    sidx_in = nc.dram_tensor("sidx", [128, gtot // 16], I16, kind="ExternalInput")
    cvec_in = nc.dram_tensor("cvec", [ncv * 128, H], FP, kind="ExternalInput")
    bn_in = nc.dram_tensor("bnvec", [128, 10], FP, kind="ExternalInput")
    w_in = [nc.dram_tensor(f"w{i}", [H, H], FP, kind="ExternalInput")
            for i in range(5)]
    out_ext = nc.dram_tensor("out", [npad, H], FP, kind="ExternalOutput")

    # const vector order in cvec
    CV_C0 = 0
    def CV_G(i): return 1 + 3 * i
    def CV_BT(i): return 2 + 3 * i
    def CV_BTC(i): return 3 + 3 * i   # only i<3

    with tile.TileContext(nc) as tc:
        with (
            tc.tile_pool(name="const", bufs=1) as cp,
            tc.tile_pool(name="msg", bufs=2) as mp,
            tc.tile_pool(name="nodeio", bufs=2) as np_io,
            tc.tile_pool(name="nodetmp", bufs=1) as np_tmp,
            tc.tile_pool(name="stats", bufs=2) as sp,
            tc.tile_pool(name="psA", bufs=4, space="PSUM") as psA,
            tc.tile_pool(name="psV", bufs=2, space="PSUM") as psV,
            tc.tile_pool(name="dram", bufs=1, space="DRAM") as dp,
        ):
            nc.gpsimd.load_library(library_config.mlp)

            reg_cache = {}
            def sreg(v):
                if v not in reg_cache:
                    reg_cache[v] = nc.gpsimd.to_reg(v)
                return reg_cache[v]

            # ---- persistent SBUF state
            ident = cp.tile([128, 128], FP)
            make_identity(nc, ident[:])
            codes_sb = cp.tile([128, npad // 16], I16)
            nc.sync.dma_start(codes_sb[:], codes_in[:])
            gidx_sb = cp.tile([128, gtot // 16], I16)
            nc.sync.dma_start(gidx_sb[:], gidx_in[:])
            sidx_sb = cp.tile([128, gtot // 16], I16)
            nc.sync.dma_start(sidx_sb[:], sidx_in[:])
            cvec_sb = cp.tile([128, ncv, H], FP)
            nc.sync.dma_start(cvec_sb[:], cvec_in.rearrange("(v p) h -> p v h", p=128))
            bn_sb = cp.tile([128, 10], FP)
            nc.sync.dma_start(bn_sb[:], bn_in[:])
            w_sb = []
            for i in range(5):
                w = cp.tile([128, 2, H], FP, name=f"w_sb{i}")
                nc.sync.dma_start(w[:], w_in[i].rearrange("(b p) h -> p b h", p=128))
                w_sb.append(w)

            def cvape(v):   # [128, 1, H] const vec -> broadcast over node axis
                return cvec_sb[:, v:v + 1, :].to_broadcast([128, nt, H])

            # ---- DRAM internals
            m_loc = dp.tile([npad, H], FP)
            m_fulls = [dp.tile([nfull, H], FP, addr_space="Shared",
                               name=f"m_full{i}") for i in range(4)]
            agg = [dp.tile([npad + trash, H], FP, name=f"agg{j}") for j in (0, 1)]

            def node_rows(t_dram, base, rows):
                return t_dram[base:base + rows, :].rearrange(
                    "(x p) h -> p x h", p=128)

            # zero the trash rows once
            zt = mp.tile([128, trash // 128, H], FP, tag="msg")
            nc.gpsimd.memset(zt[:], 0.0)
            for j in (0, 1):
                nc.sync.dma_start(node_rows(agg[j], npad, trash), zt[:])

            # ---- L0: LUT gather -> h0 (agg[0] init) and m0
            off16 = 0
            base = 0
            for (sz, valid) in l0_grid:
                t = mp.tile([128, cmax, H], FP, tag="msg")
                if valid < sz:
                    nc.gpsimd.memset(t[:, :sz // 128, :], 0.0)
                nc.gpsimd.dma_gather(
                    t[:, :sz // 128, :], lut_in[:], codes_sb[:, off16:off16 + sz // 16],
                    sz, sreg(valid), H)
                nc.sync.dma_start(node_rows(agg[0], base, sz), t[:, :sz // 128, :])
                m0 = mp.tile([128, cmax, H], FP, tag="m0")
                nc.vector.tensor_tensor(
                    out=m0[:, :sz // 128, :], in0=t[:, :sz // 128, :],
                    in1=cvec_sb[:, CV_C0:CV_C0 + 1, :].to_broadcast([128, sz // 128, H]),
                    op=ALU.add)
                nc.scalar.activation(m0[:, :sz // 128, :], m0[:, :sz // 128, :], AF.Relu)
                nc.sync.dma_start(node_rows(m_loc, base, sz), m0[:, :sz // 128, :])
                off16 += sz // 16
                base += sz

            # ---- layers
            for li in range(4):
                cur, nxt = agg[li % 2], agg[(li + 1) % 2]

                m_full = m_fulls[li]
                nc.gpsimd.collective_compute(
                    "AllGather", ALU.bypass,
                    replica_groups=[list(range(ncore))],
                    ins=[m_loc[:]], outs=[m_full[:]])

                # edge phase
                goff16 = 0
                for (b, rows_span, sizes) in edge_grid:
                    src_ap = m_full[b * blk_rows: b * blk_rows + rows_span, :]
                    for sz in sizes:
                        msg = mp.tile([128, cmax, H], FP, tag="msg")
                        nc.gpsimd.dma_gather(
                            msg[:, :sz // 128, :], src_ap,
                            gidx_sb[:, goff16:goff16 + sz // 16], sz, sreg(sz), H)
                        nc.gpsimd.dma_scatter_add(
                            cur[:], msg[:, :sz // 128, :],
                            sidx_sb[:, goff16:goff16 + sz // 16], sz, sreg(sz), H)
                        goff16 += sz // 16

                # node phase
                last = li == 3
                for t in range(n_tiles):
                    rows0 = t * node_tile
                    x = np_io.tile([128, nt, H], FP, tag="x")
                    nc.sync.dma_start(x[:], node_rows(cur, rows0, node_tile))

                    xT = np_tmp.tile([128, 2, node_tile], FP, tag="xT")
                    for j in range(nt):
                        for k in range(2):
                            tp = psA.tile([128, 128], FP, tag="tp")
                            nc.tensor.transpose(
                                tp[:], x[:, j, k * 128:(k + 1) * 128], ident[:])
                            nc.vector.tensor_copy(
                                xT[:, k, j * 128:(j + 1) * 128], tp[:])

                    yT = np_tmp.tile([128, 2, node_tile], FP, tag="yT")
                    for f in range(2):
                        v = psV.tile([128, node_tile], FP, tag="v")
                        for k in range(2):
                            nc.tensor.matmul(
                                v[:], w_sb[li][:, k, f * 128:(f + 1) * 128],
                                xT[:, k, :], start=(k == 0), stop=(k == 1))
                        nc.vector.tensor_scalar(
                            out=yT[:, f, :], in0=v[:],
                            scalar1=bn_sb[:, 2 * li + f:2 * li + f + 1],
                            scalar2=None, op0=ALU.add)

                    y = np_tmp.tile([128, nt, H], FP, tag="y")
                    for j in range(nt):
                        for f in range(2):
                            tp = psA.tile([128, 128], FP, tag="tp")
                            nc.tensor.transpose(
                                tp[:], yT[:, f, j * 128:(j + 1) * 128], ident[:])
                            nc.vector.tensor_copy(
                                y[:, j, f * 128:(f + 1) * 128], tp[:])

                    # LN stats per node (over H)
                    s = sp.tile([128, nt, 1], FP, tag="s")
                    nc.vector.reduce_sum(out=s[:], in_=y[:], axis=mybir.AxisListType.X)
                    sq = np_tmp.tile([128, nt, H], FP, tag="sq")
                    nc.vector.tensor_tensor(out=sq[:], in0=y[:], in1=y[:], op=ALU.mult)
                    q = sp.tile([128, nt, 1], FP, tag="q")
                    nc.vector.reduce_sum(out=q[:], in_=sq[:], axis=mybir.AxisListType.X)
                    mu = sp.tile([128, nt, 1], FP, tag="mu")
                    nc.vector.tensor_scalar(
                        out=mu[:], in0=s[:], scalar1=1.0 / H, scalar2=None, op0=ALU.mult)
                    vr = sp.tile([128, nt, 1], FP, tag="vr")
                    nc.vector.tensor_scalar(
                        out=vr[:], in0=q[:], scalar1=1.0 / H, scalar2=None, op0=ALU.mult)
                    m2 = sp.tile([128, nt, 1], FP, tag="m2")
                    nc.vector.tensor_tensor(out=m2[:], in0=mu[:], in1=mu[:], op=ALU.mult)
                    nc.vector.tensor_tensor(out=vr[:], in0=vr[:], in1=m2[:], op=ALU.subtract)
                    nc.vector.tensor_scalar(
                        out=vr[:], in0=vr[:], scalar1=LN_EPS, scalar2=None, op0=ALU.add)
                    rv = sp.tile([128, nt, 1], FP, tag="rv")
                    nc.vector.reciprocal(rv[:], vr[:])
                    A = sp.tile([128, nt, 1], FP, tag="A")
                    nc.scalar.activation(A[:], rv[:], AF.Sqrt)
                    B = sp.tile([128, nt, 1], FP, tag="B")
                    nc.vector.tensor_tensor(out=B[:], in0=mu[:], in1=A[:], op=ALU.mult)

                    # u = ((y - mu) * rstd) * g   (into sq's buffer, then y's)
                    tn = sq
                    nc.vector.tensor_tensor(
                        out=tn[:], in0=y[:], in1=A[:].to_broadcast([128, nt, H]),
                        op=ALU.mult)
                    nc.vector.tensor_tensor(
                        out=tn[:], in0=tn[:], in1=B[:].to_broadcast([128, nt, H]),
                        op=ALU.subtract)
                    u = y
                    nc.vector.tensor_tensor(
                        out=u[:], in0=tn[:], in1=cvape(CV_G(li)), op=ALU.mult)

                    if not last:
                        hn = np_io.tile([128, nt, H], FP, tag="hn")
                        nc.vector.tensor_tensor(
                            out=hn[:], in0=u[:], in1=cvape(CV_BT(li)), op=ALU.add)
                        nc.sync.dma_start(node_rows(nxt, rows0, node_tile), hn[:])
                        mn = np_io.tile([128, nt, H], FP, tag="mn")
                        nc.vector.tensor_tensor(
                            out=mn[:], in0=u[:], in1=cvape(CV_BTC(li)), op=ALU.add)
                        nc.scalar.activation(mn[:], mn[:], AF.Relu)
                        nc.sync.dma_start(node_rows(m_loc, rows0, node_tile), mn[:])
                    else:
                        ln = np_tmp.tile([128, nt, H], FP, tag="ln")
                        nc.vector.tensor_tensor(
                            out=ln[:], in0=u[:], in1=cvape(CV_BT(li)), op=ALU.add)
                        lnT = np_tmp.tile([128, 2, node_tile], FP, tag="lnT")
                        for j in range(nt):
                            for k in range(2):
                                tp = psA.tile([128, 128], FP, tag="tp")
                                nc.tensor.transpose(
                                    tp[:], ln[:, j, k * 128:(k + 1) * 128], ident[:])
                                nc.vector.tensor_copy(
                                    lnT[:, k, j * 128:(j + 1) * 128], tp[:])
                        zT = np_tmp.tile([128, 2, node_tile], FP, tag="zT")
                        for f in range(2):
                            v2 = psV.tile([128, node_tile], FP, tag="v")
                            for k in range(2):
                                nc.tensor.matmul(
                                    v2[:], w_sb[4][:, k, f * 128:(f + 1) * 128],
                                    lnT[:, k, :], start=(k == 0), stop=(k == 1))
                            nc.vector.tensor_scalar(
                                out=zT[:, f, :], in0=v2[:],
                                scalar1=bn_sb[:, 8 + f:9 + f],
                                scalar2=None, op0=ALU.add)
                        z = np_io.tile([128, nt, H], FP, tag="z")
                        for j in range(nt):
                            for f in range(2):
                                tp = psA.tile([128, 128], FP, tag="tp")
                                nc.tensor.transpose(
                                    tp[:], zT[:, f, j * 128:(j + 1) * 128], ident[:])
                                nc.vector.tensor_copy(
                                    z[:, j, f * 128:(f + 1) * 128], tp[:])
                        nc.sync.dma_start(node_rows(out_ext, rows0, node_tile), z[:])

    _finalize_bir(nc)
    return nc


# ---------------------------------------------------------------------------
# entry point
# ---------------------------------------------------------------------------

def _prep_and_run(inputs, ncore, nloc, npad, node_tile, trace=False):
    atom = np.asarray(inputs["atom_inputs"])
    src = np.asarray(inputs["src"]).astype(np.int64)
    dst = np.asarray(inputs["dst"]).astype(np.int64)
    nfull = ncore * npad

    # LUT for AtomEmbedding + linear_0
    tuples, codes = np.unique(np.asarray(atom, np.int64), axis=0,
                              return_inverse=True)
    feats = np.concatenate([
        np.asarray(inputs["emb_element"])[tuples[:, 0]],
        np.asarray(inputs["emb_degree"])[tuples[:, 1]],
        np.asarray(inputs["emb_valence"])[tuples[:, 2] + 1],
        np.asarray(inputs["emb_charge"])[tuples[:, 3]],
        np.asarray(inputs["emb_aromatic"])[tuples[:, 4]],
        np.asarray(inputs["emb_hybrid"])[tuples[:, 5]],
        np.asarray(inputs["emb_hydrogen"])[tuples[:, 6]],
    ], axis=-1).astype(np.float32)
    lut = (feats @ np.asarray(inputs["W0"], np.float32)
           + np.asarray(inputs["b0"], np.float32)).astype(np.float32)
    assert len(lut) < 32768

    # per-feature const vectors
    cv = np.zeros((12, H), np.float32)
    c_layer = [np.asarray(inputs[f"We{i}"], np.float32)
               + np.asarray(inputs[f"be{i}"], np.float32) for i in range(4)]
    cv[0] = c_layer[0]
    for i in range(4):
        cv[1 + 3 * i] = np.asarray(inputs[f"g{i}"], np.float32)
        cv[2 + 3 * i] = np.asarray(inputs[f"bt{i}"], np.float32)
        if i < 3:
            cv[3 + 3 * i] = np.asarray(inputs[f"bt{i}"], np.float32) + c_layer[i + 1]
    cvec = np.broadcast_to(cv[:, None, :], (12, 128, H)).reshape(12 * 128, H)
    cvec = np.ascontiguousarray(cvec, np.float32)

    bnvec = np.zeros((128, 10), np.float32)
    for i in range(4):
        bn = np.asarray(inputs[f"bn{i}"], np.float32)
        bnvec[:, 2 * i] = bn[:128]
        bnvec[:, 2 * i + 1] = bn[128:]
    b1 = np.asarray(inputs["b1"], np.float32)
    bnvec[:, 8] = b1[:128]
    bnvec[:, 9] = b1[128:]

    weights = [np.ascontiguousarray(np.asarray(inputs[f"Wn{i}"], np.float32))
               for i in range(4)]
    weights.append(np.ascontiguousarray(np.asarray(inputs["W1"], np.float32)))

    # L0 grid
    l0_grid = []
    base = 0
    while base < npad:
        sz = min(CH_L0, npad - base)
        valid = max(0, min(nloc - base, sz))
        assert valid > 0
        l0_grid.append((sz, valid))
        base += sz

    edge_grid, gidx_bycore, sidx_bycore = _prep_edges(
        src, dst, nloc, npad, BLK, ncore, CH_EDGE, TRASH)
    gtot = sum(sz for (_, _, sizes) in edge_grid for sz in sizes)

    cfg = dict(ncore=ncore, nloc=nloc, npad=npad, nfull=nfull,
               lutrows=len(lut), l0_grid=l0_grid, edge_grid=edge_grid,
               gtot=gtot, blk_rows=BLK, node_tile=node_tile, trash=TRASH)
    nc = build_program(cfg)

    in_maps = []
    for k in range(ncore):
        codes_k = np.full(npad, -1, np.int64)
        codes_k[:nloc] = codes[k * nloc:(k + 1) * nloc]
        im = {
            "lut": lut,
            "codes": _wrap16(codes_k),
            "gidx": _wrap16(gidx_bycore[k]),
            "sidx": _wrap16(sidx_bycore[k]),
            "cvec": cvec,
            "bnvec": bnvec,
        }
        for i in range(5):
            im[f"w{i}"] = weights[i]
        in_maps.append(im)

    res = run_bass_kernel_spmd(nc, in_maps, list(range(ncore)), trace=trace)
    out = np.concatenate(
        [res.results[k]["out"][:nloc] for k in range(ncore)], axis=0)
    return out, res


def kernel(**inputs) -> np.ndarray:
    trace = bool(int(os.environ.get("BASS_GNN_TRACE", "0")))
    if trace:
        try:
            import sys, types
            if "antenv.axon_hooks" not in sys.modules:
                mod = types.ModuleType("antenv.axon_hooks")
                _h = [None]
                mod.set_axon_ntff_profile_hook = lambda h: _h.__setitem__(0, h)
                mod.get_axon_ntff_profile_hook = lambda: _h[0]
                import antenv
                sys.modules["antenv.axon_hooks"] = mod
                antenv.axon_hooks = mod
                from trn_agent_boot.trn_boot import _ntff_profile_via_ctypes
                mod.set_axon_ntff_profile_hook(
                    _ntff_profile_via_ctypes("/opt/axon/libaxon_pjrt.so"))
        except Exception as e:
            print("trace hook setup failed:", e)
            trace = False
    out, res = _prep_and_run(inputs, NCORE, NLOC, NPAD, NODE_TILE, trace=trace)
    if trace and res.exec_time_ns is not None:
        print(f"HW exec time: {res.exec_time_ns} ns")
    return out

